# revision 43
# baseline (speedup 1.0000x reference)
"""Linear-attention Trainium2 Bass kernel (optimized).

Reference computation (per batch b, head h):
    qkv = x @ W^T                         (t, 3072)
    q,k,v -> (h, t, 64)
    k masked rows -> -inf; prepend 4 mem-kv rows
    q = softmax(q * d^-0.5, axis=feature)
    k = softmax(k, axis=sequence)
    ctx = k^T v   (64x64);  out = q @ ctx;  out *= mask

Key optimizations over the naive mapping:
  * Masked tokens (~50%) contribute nothing to ctx (their k-softmax weight is
    0) and their output rows are zeroed; the host gathers only unmasked tokens
    and pads to NTOK=2304, nearly halving all projection work.
  * q/k projections run in fp8e4m3 with DoubleRow perf mode (256-row
    contraction per PE pass = 2x bf16 throughput); weights are pre-scaled by
    32 so fp8 keeps mantissa bits, and the 1/32 is folded into the exp
    epilogues.  v must stay bf16: softmax cancellation protects q/k from fp8
    noise but the v path has no such cancellation (fp8 anywhere in v costs
    ~2.5% output error).
  * k-softmax denominator = ones-column appended to v in the context matmul;
    padding rows are killed by a -1e30 exp bias.
  * Pass B emits the output TRANSPOSED ([head-col, token]) and exp(q) is
    DMAed out during pass A; the q-softmax denominator + divide happen on the
    host, so pass B is pure matmul + cast.
  * Dummy matmuls bridge PE-idle windows (startup DMA, ctx finalize) to keep
    the HAM clock-gate at full speed.

Sharding: 8 cores = (batch 0..3) x (head-half 0..1); no cross-core traffic.
"""

import numpy as np

D_MODEL = 1024
N_HEADS = 16
D_HEAD = 64
NMEM = 4
SCALE = D_HEAD ** -0.5
B = 4
L = 4096
NCORES = 8
HPC = 8            # heads per core
NPAIR = HPC // 2   # head-pairs per core
ECOLS = HPC * D_HEAD  # 512 output columns per core
NDB = D_MODEL // 128  # 8 contraction blocks

NTOK = 2176        # padded gathered-token capacity (>= max unmasked per batch;
                   # binom(4096,1/2) makes >2176 a ~4-sigma event, and kernel()
                   # falls back to a larger rebuild if an input ever exceeds it)
MODE = "fp8"       # "fp8": q/k in fp8 DoubleRow, v in bf16.  "bf16": all bf16.
WS = 32.0          # weight pre-scale for fp8 (folded back in epilogues)

_CACHE = {}


def _chunks(ntok):
    out = []
    c0 = 0
    while c0 < ntok:
        cw = min(512, ntok - c0)
        out.append((c0, cw))
        c0 += cw
    return out


def build_nc(ntok=NTOK, mode=MODE):
    """Build the per-core Bass program (identical across cores; data differs)."""
    import concourse.tile as tile
    from concourse import bacc, mybir

    f32 = mybir.dt.float32
    bf16 = mybir.dt.bfloat16
    AF = mybir.ActivationFunctionType
    DR = mybir.MatmulPerfMode.DoubleRow

    fp8 = mode == "fp8"
    xdt = mybir.dt.float8e4 if fp8 else bf16
    ws = WS if fp8 else 1.0

    n_tb = ntok // 128
    chunks = _chunks(ntok)

    nc = bacc.Bacc("TRN2", target_bir_lowering=False, debug=False)

    # x streams are laid out CHUNK-MAJOR on the host ([p, chunk|(db t)]) so each
    # per-chunk DMA is one contiguous 4KB-per-partition block (the naive
    # [d_model, t] slice produces 512B blocks that run at ~1/3 DMA bandwidth)
    xT = nc.dram_tensor("xT", (128, NDB * ntok), xdt, kind="ExternalInput").ap()
    if fp8:  # v path stays bf16 (fp8 wv/x injects ~2.5% output error)
        xTb = nc.dram_tensor("xTb", (128, NDB * ntok), bf16, kind="ExternalInput").ap()
    wq = nc.dram_tensor("wq", (128, NDB, ECOLS), xdt, kind="ExternalInput").ap()
    wk = nc.dram_tensor("wk", (128, NDB, ECOLS), xdt, kind="ExternalInput").ap()
    wv = nc.dram_tensor("wv", (128, NDB, ECOLS), bf16, kind="ExternalInput").ap()
    mkp = nc.dram_tensor("mkp", (NPAIR, NMEM, 128), bf16, kind="ExternalInput").ap()
    mvp = nc.dram_tensor("mvp", (NPAIR, NMEM, 130), bf16, kind="ExternalInput").ap()
    biasm = nc.dram_tensor("biasm", (128, n_tb), f32, kind="ExternalInput").ap()
    outT = nc.dram_tensor("outT", (NPAIR, 128, ntok), bf16, kind="ExternalOutput").ap()
    expqd = nc.dram_tensor("expqd", (128, NPAIR, ntok), bf16, kind="ExternalOutput").ap()

    with tile.TileContext(nc) as tc:
        with (
            tc.tile_pool(name="const", bufs=1) as cpool,
            tc.tile_pool(name="big", bufs=1) as bigpool,
            tc.tile_pool(name="small", bufs=8) as small,
            tc.tile_pool(name="xt", bufs=2) as xt_pool,
            tc.tile_pool(name="xtb", bufs=2) as xtb_pool,
            tc.tile_pool(name="ek", bufs=5) as ek_pool,
            tc.tile_pool(name="vv", bufs=3) as vv_pool,
            tc.tile_pool(name="osb", bufs=4) as osb_pool,
        ):
            # ---- PE warm-up: dense dummy matmuls while DMAs land ----
            with (
                tc.tile_pool(name="warm", bufs=1) as warm_pool,
                tc.tile_pool(name="warmps", bufs=1, space="PSUM") as warmps_pool,
            ):
                # sized to span the initial weight/x DMA window (~5-6us at the
                # cold 1.2 GHz clock) so the PE never goes idle long enough
                # for the HAM monitor to re-throttle before pass A starts
                wrm = warm_pool.tile([128, 512], bf16, name="wrm", tag="wrm")
                nc.vector.memset(wrm, 0.0)
                wps = warmps_pool.tile([128, 512], f32, name="wps", tag="wps")
                for i in range(5):
                    nc.tensor.matmul(
                        wps, lhsT=wrm[:, 0:128], rhs=wrm,
                        start=(i == 0), stop=(i == 4),
                    )

            # ---- constants / weights ----
            w_sbs = []
            for nm, w_dram, wdt in (("wq", wq, xdt), ("wk", wk, xdt), ("wv", wv, bf16)):
                w_sb = cpool.tile([128, NDB, ECOLS], wdt, name=f"{nm}_sb", tag=f"{nm}_sb")
                nc.sync.dma_start(out=w_sb, in_=w_dram)
                w_sbs.append(w_sb)
            wq_sb, wk_sb, wv_sb = w_sbs

            mk_sb = cpool.tile([NMEM, NPAIR * 128], bf16, name="mk_sb", tag="mk_sb")
            nc.sync.dma_start(
                out=mk_sb.rearrange("n (g d) -> n g d", g=NPAIR),
                in_=mkp.rearrange("g n d -> n g d"),
            )
            mv_sb = cpool.tile([NMEM, NPAIR * 130], bf16, name="mv_sb", tag="mv_sb")
            nc.sync.dma_start(
                out=mv_sb.rearrange("n (g e) -> n g e", g=NPAIR),
                in_=mvp.rearrange("g n e -> n g e"),
            )

            biasm_sb = cpool.tile([128, n_tb], f32, name="biasm_sb", tag="biasm_sb")
            nc.sync.dma_start(out=biasm_sb, in_=biasm)

            # exp(q * scale) for the whole batch, kept resident: [128, pair, t]
            expq_sb = bigpool.tile([128, NPAIR, ntok], bf16, name="expq_sb", tag="expq_sb")

            with tc.tile_pool(name="ctxps", bufs=1, space="PSUM") as ctx_pool:
                # persistent context accumulators (one psum bank per pair)
                ctx_ps = [
                    ctx_pool.tile([128, 130], f32, name=f"ctx_ps{i}", tag=f"ctx{i}")
                    for i in range(NPAIR)
                ]

                # ---- mem-kv rows initialize the context accumulation ----
                for g in range(NPAIR):
                    nc.tensor.matmul(
                        ctx_ps[g],
                        lhsT=mk_sb[:, g * 128 : (g + 1) * 128],
                        rhs=mv_sb[:, g * 130 : (g + 1) * 130],
                        start=True,
                        stop=False,
                    )

                # ---- pass A: projection + exp + context accumulation ----
                with (
                    tc.tile_pool(name="pq", bufs=2, space="PSUM") as pq_pool,
                    tc.tile_pool(name="pk", bufs=1, space="PSUM") as pk_pool,
                    tc.tile_pool(name="pv", bufs=1, space="PSUM") as pv_pool,
                ):
                    for ci, (c0, cw) in enumerate(chunks):
                        # x streams ride the Activation HWDGE queue so they
                        # don't serialize behind weight/output DMAs on SP.
                        xt = xt_pool.tile([128, NDB, cw], xdt, name="xt")
                        nc.scalar.dma_start(
                            out=xt,
                            in_=xT[:, NDB * c0 : NDB * (c0 + cw)].rearrange(
                                "p (db t) -> p db t", db=NDB
                            ),
                        )
                        if fp8:
                            xtb = xtb_pool.tile([128, NDB, cw], bf16, name="xtb")
                            nc.scalar.dma_start(
                                out=xtb,
                                in_=xTb[:, NDB * c0 : NDB * (c0 + cw)].rearrange(
                                    "p (db t) -> p db t", db=NDB
                                ),
                            )
                        else:
                            xtb = xt

                        # qT projection per pair: psum [128 = pair-dheads, cw]
                        for g in range(NPAIR):
                            pq = pq_pool.tile([128, cw], f32, name="pq")
                            if fp8:
                                for s in range(NDB // 2):
                                    nc.tensor.matmul(
                                        pq,
                                        lhsT=wq_sb[:, 2 * s : 2 * s + 2, g * 128 : (g + 1) * 128],
                                        rhs=xt[:, 2 * s : 2 * s + 2, :],
                                        start=(s == 0),
                                        stop=(s == NDB // 2 - 1),
                                        perf_mode=DR,
                                    )
                            else:
                                for db in range(NDB):
                                    nc.tensor.matmul(
                                        pq,
                                        lhsT=wq_sb[:, db, g * 128 : (g + 1) * 128],
                                        rhs=xt[:, db, :],
                                        start=(db == 0),
                                        stop=(db == NDB - 1),
                                    )
                            nc.scalar.activation(
                                expq_sb[:, g, c0 : c0 + cw], pq, AF.Exp, scale=SCALE / ws
                            )
                        # ship exp(q) to the host (q-softmax denominator +
                        # divide happen host-side; pass B stays matmul-dense)
                        nc.sync.dma_start(
                            out=expqd[:, :, c0 : c0 + cw],
                            in_=expq_sb[:, :, c0 : c0 + cw],
                        )

                        # k/v projection + exp(k)+padkill + context, per t-block
                        for tbi in range(cw // 128):
                            j = c0 // 128 + tbi
                            t0, t1 = tbi * 128, (tbi + 1) * 128
                            pk = pk_pool.tile([128, ECOLS], f32, name="pk")
                            pv = pv_pool.tile([128, ECOLS], f32, name="pv")
                            if fp8:
                                for s in range(NDB // 2):
                                    nc.tensor.matmul(
                                        pk,
                                        lhsT=xt[:, 2 * s : 2 * s + 2, t0:t1],
                                        rhs=wk_sb[:, 2 * s : 2 * s + 2, :],
                                        start=(s == 0),
                                        stop=(s == NDB // 2 - 1),
                                        perf_mode=DR,
                                    )
                            else:
                                for db in range(NDB):
                                    nc.tensor.matmul(
                                        pk,
                                        lhsT=xt[:, db, t0:t1],
                                        rhs=wk_sb[:, db, :],
                                        start=(db == 0),
                                        stop=(db == NDB - 1),
                                    )
                            for db in range(NDB):
                                nc.tensor.matmul(
                                    pv,
                                    lhsT=xtb[:, db, t0:t1],
                                    rhs=wv_sb[:, db, :],
                                    start=(db == 0),
                                    stop=(db == NDB - 1),
                                )
                            ek = ek_pool.tile([128, ECOLS], bf16, name="ek")
                            nc.scalar.activation(
                                ek, pk, AF.Exp, bias=biasm_sb[:, j : j + 1], scale=1.0 / ws
                            )
                            vv = vv_pool.tile([128, NPAIR, 130], bf16, name="vv")
                            nc.vector.tensor_copy(
                                vv[:, :, 0:128],
                                pv.rearrange("p (g e) -> p g e", g=NPAIR),
                            )
                            nc.vector.memset(vv[:, :, 128:130], 1.0)
                            for g in range(NPAIR):
                                nc.tensor.matmul(
                                    ctx_ps[g],
                                    lhsT=ek[:, g * 128 : (g + 1) * 128],
                                    rhs=vv[:, g, :],
                                    start=False,
                                    stop=(j == n_tb - 1),
                                )
                            last_vv = vv

                    # Dummy matmuls keep the PE busy across the vector-serial
                    # finalize below so the HAM activity monitor doesn't
                    # re-throttle the clock for pass B.  Reading the LAST vv
                    # tile pins their schedule to the end of pass A (operands
                    # with no late deps would let the scheduler hoist them
                    # into mid-pass A); emitting them inside this pool scope
                    # avoids a pool-close barrier in front of them.
                    vvf = last_vv.rearrange("p g e -> p (g e)")
                    kps = pq_pool.tile([128, 512], f32, name="pq")
                    for i in range(14):
                        nc.tensor.matmul(
                            kps, lhsT=vvf[:, 0:128], rhs=vvf[:, 0:512],
                            start=(i == 0), stop=(i == 13),
                        )

                # ---- finalize: normalize ctx into block-diagonal lhsT ----
                ctxbd = cpool.tile([128, NPAIR * 128], bf16, name="ctxbd", tag="ctxbd")
                nc.vector.memset(ctxbd, 0.0)
                for g in range(NPAIR):
                    ps = ctx_ps[g]
                    rk = small.tile([128, 1], f32, name="rk", tag="rk")
                    nc.vector.reciprocal(rk, ps[:, 128:129])
                    o = g * 128
                    nc.vector.tensor_scalar_mul(
                        ctxbd[0:64, o : o + 64], ps[0:64, 0:64], rk[0:64]
                    )
                    nc.vector.tensor_scalar_mul(
                        ctxbd[64:128, o + 64 : o + 128], ps[64:128, 64:128], rk[64:128]
                    )

            # ---- pass B: outT[e, t] = (ctxn^T expq)[e, t] per pair ----
            # outT DMAs alternate between the two HWDGE queues: ~0.6us of
            # descriptor generation per DMA paces the pipeline if they all
            # share one queue (and a batched pair-major DMA is far worse --
            # its partition-crossing pattern costs ~4us of desc-gen each)
            with tc.tile_pool(name="po", bufs=6, space="PSUM") as po_pool:
                for ci, (c0, cw) in enumerate(chunks):
                    for g in range(NPAIR):
                        po = po_pool.tile([128, cw], f32, name="po")
                        nc.tensor.matmul(
                            po,
                            lhsT=ctxbd[:, g * 128 : (g + 1) * 128],
                            rhs=expq_sb[:, g, c0 : c0 + cw],
                            start=True,
                            stop=True,
                        )
                        osb = osb_pool.tile([128, cw], bf16, name="osb")
                        if g % 2 == 0:
                            nc.vector.tensor_copy(osb, po)
                            nc.sync.dma_start(out=outT[g, :, c0 : c0 + cw], in_=osb)
                        else:
                            nc.scalar.activation(osb, po, AF.Copy)
                            nc.scalar.dma_start(out=outT[g, :, c0 : c0 + cw], in_=osb)

    nc.compile()
    return nc


def _host_inputs(x, w_qkv, mem_kv, mask, ntok=NTOK, mode=MODE):
    """Build the 8 per-core input maps on the host; returns (maps, idx list)."""
    import ml_dtypes

    fp8 = mode == "fp8"
    xnp = ml_dtypes.float8_e4m3 if fp8 else ml_dtypes.bfloat16
    ws = WS if fp8 else 1.0

    x = np.asarray(x, dtype=np.float32)
    w_qkv = np.asarray(w_qkv, dtype=np.float32)
    mem_kv = np.asarray(mem_kv, dtype=np.float32)
    mask = np.asarray(mask)

    nb = x.shape[0]
    n_tb = ntok // 128

    idxs, xTs, xTbs, biasms = [], [], [], []
    for b in range(nb):
        idx = np.nonzero(mask[b])[0]
        n = len(idx)
        assert n <= ntok, f"unmasked tokens {n} > capacity {ntok}"
        idxs.append(idx)
        xg = np.zeros((ntok, D_MODEL), np.float32)
        xg[:n] = x[b][idx]
        # chunk-major layout [128, sum_c(NDB*cw)]: element
        # [p, NDB*c0 + db*cw + t] = x[db*128+p, c0+t] -> each per-chunk DMA
        # reads one contiguous 4KB-per-partition block
        xc = np.empty((128, NDB * ntok), np.float32)
        for c0, cw in _chunks(ntok):
            blk = (
                xg[c0 : c0 + cw, :].T.reshape(NDB, 128, cw)
                .transpose(1, 0, 2)
                .reshape(128, NDB * cw)
            )
            xc[:, NDB * c0 : NDB * (c0 + cw)] = blk
        xTs.append(xc.astype(xnp))
        if fp8:
            xTbs.append(xc.astype(ml_dtypes.bfloat16))
        bm = np.zeros(ntok, np.float32)
        bm[n:] = -1e30
        biasms.append(np.ascontiguousarray(bm.reshape(n_tb, 128).T))

    # weights: [3072, 1024] -> per (half, proj): [128, NDB, ECOLS]
    w4 = w_qkv.reshape(N_HEADS, D_HEAD, 3, D_MODEL)
    wT = {}
    for half in (0, 1):
        h0 = half * HPC
        for ci, cn in ((0, "q"), (1, "k"), (2, "v")):
            wdt = ml_dtypes.bfloat16 if cn == "v" else xnp
            wsc = 1.0 if cn == "v" else ws
            w2 = w4[h0 : h0 + HPC, :, ci, :].reshape(ECOLS, D_MODEL) * wsc
            # [cols, d] -> [p, db, cols] with d = db*128 + p
            wT[(half, cn)] = np.ascontiguousarray(
                w2.T.reshape(NDB, 128, ECOLS).transpose(1, 0, 2)
            ).astype(wdt)

    in_maps = []
    for c in range(NCORES):
        b, half = divmod(c, 2)
        h0 = half * HPC
        mk = (
            mem_kv[0, h0 : h0 + HPC]
            .reshape(NPAIR, 2, NMEM, D_HEAD)
            .transpose(0, 2, 1, 3)
            .reshape(NPAIR, NMEM, 128)
        )
        mv = (
            mem_kv[1, h0 : h0 + HPC]
            .reshape(NPAIR, 2, NMEM, D_HEAD)
            .transpose(0, 2, 1, 3)
            .reshape(NPAIR, NMEM, 128)
        )
        mvp = np.ones((NPAIR, NMEM, 130), np.float32)
        mvp[:, :, :128] = mv
        im = {
            "xT": xTs[b],
            "wq": wT[(half, "q")],
            "wk": wT[(half, "k")],
            "wv": wT[(half, "v")],
            "mkp": np.exp(mk).astype(ml_dtypes.bfloat16),
            "mvp": mvp.astype(ml_dtypes.bfloat16),
            "biasm": biasms[b],
        }
        if fp8:
            im["xTb"] = xTbs[b]
        in_maps.append(im)
    return in_maps, idxs


def _assemble(results, idxs, nb=B, seqlen=L):
    """Divide num/den, transpose, scatter into the full (b, l, d) output."""
    out = np.zeros((nb, seqlen, D_MODEL), np.float32)
    for c in range(NCORES):
        b, half = divmod(c, 2)
        idx = idxs[b]
        n = len(idx)
        oT = np.asarray(results[c]["outT"]).astype(np.float32)  # [4, 128, ntok]
        eq = np.asarray(results[c]["expqd"]).astype(np.float32)  # [128, 4, ntok]
        # den[g, h, t] = sum_d expq[h*64+d, g, t]
        den = eq[:, :, :n].reshape(2, 64, NPAIR, n).sum(axis=1)  # [2, 4, n]
        num = oT[:, :, :n].reshape(NPAIR, 2, 64, n)
        y = num / den.transpose(1, 0, 2)[:, :, None, :]
        y = y.transpose(3, 0, 1, 2).reshape(n, ECOLS)
        out[b, idx, half * ECOLS : (half + 1) * ECOLS] = y
    return out


def _get_nc(ntok=NTOK, mode=MODE):
    key = (ntok, mode)
    if key not in _CACHE:
        _CACHE[key] = build_nc(ntok, mode)
    return _CACHE[key]


def kernel(x, w_qkv, mem_kv, mask):
    from concourse.bass_utils import run_bass_kernel_spmd

    mask = np.asarray(mask)
    ntok = NTOK
    max_n = int(mask.sum(axis=1).max())
    if max_n > ntok:  # safety net for unexpected mask densities
        ntok = -(-max_n // 128) * 128
    nc = _get_nc(ntok)
    in_maps, idxs = _host_inputs(x, w_qkv, mem_kv, mask, ntok=ntok)
    res = run_bass_kernel_spmd(nc, in_maps, core_ids=list(range(NCORES)))
    return _assemble(res.results, idxs, nb=x.shape[0], seqlen=x.shape[1])


# revision 51
# speedup vs baseline: 1.1831x; 1.1831x over previous
"""Linear-attention Trainium2 Bass kernel (optimized).

Reference computation (per batch b, head h):
    qkv = x @ W^T                         (t, 3072)
    q,k,v -> (h, t, 64)
    k masked rows -> -inf; prepend 4 mem-kv rows
    q = softmax(q * d^-0.5, axis=feature)
    k = softmax(k, axis=sequence)
    ctx = k^T v   (64x64);  out = q @ ctx;  out *= mask

Key optimizations over the naive mapping:
  * Masked tokens (~50%) contribute nothing to ctx (their k-softmax weight is
    0) and their output rows are zeroed; the host gathers only unmasked tokens
    and pads to NTOK=2304, nearly halving all projection work.
  * q/k projections run in fp8e4m3 with DoubleRow perf mode (256-row
    contraction per PE pass = 2x bf16 throughput); weights are pre-scaled by
    32 so fp8 keeps mantissa bits, and the 1/32 is folded into the exp
    epilogues.  v must stay bf16: softmax cancellation protects q/k from fp8
    noise but the v path has no such cancellation (fp8 anywhere in v costs
    ~2.5% output error).
  * k-softmax denominator = ones-column appended to v in the context matmul;
    padding rows are killed by a -1e30 exp bias.
  * Pass B emits the output TRANSPOSED ([head-col, token]) and exp(q) is
    DMAed out during pass A; the q-softmax denominator + divide happen on the
    host, so pass B is pure matmul + cast.
  * Dummy matmuls bridge PE-idle windows (startup DMA, ctx finalize) to keep
    the HAM clock-gate at full speed.

Sharding: 8 cores = (batch 0..3) x (head-half 0..1); no cross-core traffic.
"""

import numpy as np

D_MODEL = 1024
N_HEADS = 16
D_HEAD = 64
NMEM = 4
SCALE = D_HEAD ** -0.5
B = 4
L = 4096
NCORES = 8
HPC = 8            # heads per core
NPAIR = HPC // 2   # head-pairs per core
ECOLS = HPC * D_HEAD  # 512 output columns per core
NDB = D_MODEL // 128  # 8 contraction blocks

NTOK = 2176        # padded gathered-token capacity (>= max unmasked per batch;
                   # binom(4096,1/2) makes >2176 a ~4-sigma event, and kernel()
                   # falls back to a larger rebuild if an input ever exceeds it)
MODE = "fp8"       # "fp8": q/k in fp8 DoubleRow, v in bf16.  "bf16": all bf16.
WS = 32.0          # weight pre-scale for fp8 (folded back in epilogues)

_CACHE = {}


def _chunks(ntok):
    out = []
    c0 = 0
    while c0 < ntok:
        cw = min(512, ntok - c0)
        out.append((c0, cw))
        c0 += cw
    return out


def build_nc(ntok=NTOK, mode=MODE):
    """Build the per-core Bass program (identical across cores; data differs)."""
    import concourse.tile as tile
    from concourse import bacc, mybir

    f32 = mybir.dt.float32
    bf16 = mybir.dt.bfloat16
    AF = mybir.ActivationFunctionType
    DR = mybir.MatmulPerfMode.DoubleRow

    fp8 = mode == "fp8"
    xdt = mybir.dt.float8e4 if fp8 else bf16
    ws = WS if fp8 else 1.0

    n_tb = ntok // 128
    chunks = _chunks(ntok)

    nc = bacc.Bacc("TRN2", target_bir_lowering=False, debug=False)

    # x streams are laid out CHUNK-MAJOR on the host ([p, chunk|(db t)]) so each
    # per-chunk DMA is one contiguous 4KB-per-partition block (the naive
    # [d_model, t] slice produces 512B blocks that run at ~1/3 DMA bandwidth)
    xT = nc.dram_tensor("xT", (128, NDB * ntok), xdt, kind="ExternalInput").ap()
    if fp8:  # v path stays bf16 (fp8 wv/x injects ~2.5% output error)
        xTb = nc.dram_tensor("xTb", (128, NDB * ntok), bf16, kind="ExternalInput").ap()
    wq = nc.dram_tensor("wq", (128, NDB, ECOLS), xdt, kind="ExternalInput").ap()
    wk = nc.dram_tensor("wk", (128, NDB, ECOLS), xdt, kind="ExternalInput").ap()
    wv = nc.dram_tensor("wv", (128, NDB, ECOLS), bf16, kind="ExternalInput").ap()
    mkp = nc.dram_tensor("mkp", (NPAIR, NMEM, 128), bf16, kind="ExternalInput").ap()
    mvp = nc.dram_tensor("mvp", (NPAIR, NMEM, 130), bf16, kind="ExternalInput").ap()
    biasm = nc.dram_tensor("biasm", (128, n_tb), f32, kind="ExternalInput").ap()
    outT = nc.dram_tensor("outT", (NPAIR, 128, ntok), bf16, kind="ExternalOutput").ap()
    expqd = nc.dram_tensor("expqd", (128, NPAIR, ntok), bf16, kind="ExternalOutput").ap()

    with tile.TileContext(nc) as tc:
        with (
            tc.tile_pool(name="const", bufs=1) as cpool,
            tc.tile_pool(name="big", bufs=1) as bigpool,
            tc.tile_pool(name="small", bufs=8) as small,
            tc.tile_pool(name="xt", bufs=2) as xt_pool,
            tc.tile_pool(name="xtb", bufs=2) as xtb_pool,
            tc.tile_pool(name="ek", bufs=5) as ek_pool,
            tc.tile_pool(name="vv", bufs=3) as vv_pool,
            tc.tile_pool(name="osb", bufs=4) as osb_pool,
        ):
            # ---- PE warm-up: dense dummy matmuls while DMAs land ----
            with (
                tc.tile_pool(name="warm", bufs=1) as warm_pool,
                tc.tile_pool(name="warmps", bufs=1, space="PSUM") as warmps_pool,
            ):
                # sized to span the initial weight/x DMA window (~5-6us at the
                # cold 1.2 GHz clock) so the PE never goes idle long enough
                # for the HAM monitor to re-throttle before pass A starts
                wrm = warm_pool.tile([128, 512], bf16, name="wrm", tag="wrm")
                nc.vector.memset(wrm, 0.0)
                wps = warmps_pool.tile([128, 512], f32, name="wps", tag="wps")
                for i in range(9):
                    nc.tensor.matmul(
                        wps, lhsT=wrm[:, 0:128], rhs=wrm,
                        start=(i == 0), stop=(i == 8),
                    )

            # ---- constants / weights ----
            w_sbs = []
            for nm, w_dram, wdt in (("wq", wq, xdt), ("wk", wk, xdt), ("wv", wv, bf16)):
                w_sb = cpool.tile([128, NDB, ECOLS], wdt, name=f"{nm}_sb", tag=f"{nm}_sb")
                nc.sync.dma_start(out=w_sb, in_=w_dram)
                w_sbs.append(w_sb)
            wq_sb, wk_sb, wv_sb = w_sbs

            mk_sb = cpool.tile([NMEM, NPAIR * 128], bf16, name="mk_sb", tag="mk_sb")
            nc.sync.dma_start(
                out=mk_sb.rearrange("n (g d) -> n g d", g=NPAIR),
                in_=mkp.rearrange("g n d -> n g d"),
            )
            mv_sb = cpool.tile([NMEM, NPAIR * 130], bf16, name="mv_sb", tag="mv_sb")
            nc.sync.dma_start(
                out=mv_sb.rearrange("n (g e) -> n g e", g=NPAIR),
                in_=mvp.rearrange("g n e -> n g e"),
            )

            biasm_sb = cpool.tile([128, n_tb], f32, name="biasm_sb", tag="biasm_sb")
            nc.sync.dma_start(out=biasm_sb, in_=biasm)

            # exp(q * scale) for the whole batch, kept resident: [128, pair, t]
            expq_sb = bigpool.tile([128, NPAIR, ntok], bf16, name="expq_sb", tag="expq_sb")

            with tc.tile_pool(name="ctxps", bufs=1, space="PSUM") as ctx_pool:
                # persistent context accumulators (one psum bank per pair)
                ctx_ps = [
                    ctx_pool.tile([128, 130], f32, name=f"ctx_ps{i}", tag=f"ctx{i}")
                    for i in range(NPAIR)
                ]

                # ---- mem-kv rows initialize the context accumulation ----
                for g in range(NPAIR):
                    nc.tensor.matmul(
                        ctx_ps[g],
                        lhsT=mk_sb[:, g * 128 : (g + 1) * 128],
                        rhs=mv_sb[:, g * 130 : (g + 1) * 130],
                        start=True,
                        stop=False,
                    )

                # ---- pass A: projection + exp + context accumulation ----
                with (
                    tc.tile_pool(name="pq", bufs=2, space="PSUM") as pq_pool,
                    tc.tile_pool(name="pk", bufs=1, space="PSUM") as pk_pool,
                    tc.tile_pool(name="pv", bufs=1, space="PSUM") as pv_pool,
                ):
                    for ci, (c0, cw) in enumerate(chunks):
                        # x streams ride the Activation HWDGE queue so they
                        # don't serialize behind weight/output DMAs on SP.
                        xt = xt_pool.tile([128, NDB, cw], xdt, name="xt")
                        nc.scalar.dma_start(
                            out=xt,
                            in_=xT[:, NDB * c0 : NDB * (c0 + cw)].rearrange(
                                "p (db t) -> p db t", db=NDB
                            ),
                        )
                        if fp8:
                            xtb = xtb_pool.tile([128, NDB, cw], bf16, name="xtb")
                            nc.scalar.dma_start(
                                out=xtb,
                                in_=xTb[:, NDB * c0 : NDB * (c0 + cw)].rearrange(
                                    "p (db t) -> p db t", db=NDB
                                ),
                            )
                        else:
                            xtb = xt

                        # qT projection per pair: psum [128 = pair-dheads, cw]
                        for g in range(NPAIR):
                            pq = pq_pool.tile([128, cw], f32, name="pq")
                            if fp8:
                                for s in range(NDB // 2):
                                    nc.tensor.matmul(
                                        pq,
                                        lhsT=wq_sb[:, 2 * s : 2 * s + 2, g * 128 : (g + 1) * 128],
                                        rhs=xt[:, 2 * s : 2 * s + 2, :],
                                        start=(s == 0),
                                        stop=(s == NDB // 2 - 1),
                                        perf_mode=DR,
                                    )
                            else:
                                for db in range(NDB):
                                    nc.tensor.matmul(
                                        pq,
                                        lhsT=wq_sb[:, db, g * 128 : (g + 1) * 128],
                                        rhs=xt[:, db, :],
                                        start=(db == 0),
                                        stop=(db == NDB - 1),
                                    )
                            nc.scalar.activation(
                                expq_sb[:, g, c0 : c0 + cw], pq, AF.Exp, scale=SCALE / ws
                            )
                        # ship exp(q) to the host (q-softmax denominator +
                        # divide happen host-side; pass B stays matmul-dense)
                        nc.sync.dma_start(
                            out=expqd[:, :, c0 : c0 + cw],
                            in_=expq_sb[:, :, c0 : c0 + cw],
                        )

                        # k/v projection + exp(k)+padkill + context, per t-block
                        for tbi in range(cw // 128):
                            j = c0 // 128 + tbi
                            t0, t1 = tbi * 128, (tbi + 1) * 128
                            pk = pk_pool.tile([128, ECOLS], f32, name="pk")
                            pv = pv_pool.tile([128, ECOLS], f32, name="pv")
                            if fp8:
                                for s in range(NDB // 2):
                                    nc.tensor.matmul(
                                        pk,
                                        lhsT=xt[:, 2 * s : 2 * s + 2, t0:t1],
                                        rhs=wk_sb[:, 2 * s : 2 * s + 2, :],
                                        start=(s == 0),
                                        stop=(s == NDB // 2 - 1),
                                        perf_mode=DR,
                                    )
                            else:
                                for db in range(NDB):
                                    nc.tensor.matmul(
                                        pk,
                                        lhsT=xt[:, db, t0:t1],
                                        rhs=wk_sb[:, db, :],
                                        start=(db == 0),
                                        stop=(db == NDB - 1),
                                    )
                            for db in range(NDB):
                                nc.tensor.matmul(
                                    pv,
                                    lhsT=xtb[:, db, t0:t1],
                                    rhs=wv_sb[:, db, :],
                                    start=(db == 0),
                                    stop=(db == NDB - 1),
                                )
                            ek = ek_pool.tile([128, ECOLS], bf16, name="ek")
                            nc.scalar.activation(
                                ek, pk, AF.Exp, bias=biasm_sb[:, j : j + 1], scale=1.0 / ws
                            )
                            vv = vv_pool.tile([128, NPAIR, 130], bf16, name="vv")
                            nc.vector.tensor_copy(
                                vv[:, :, 0:128],
                                pv.rearrange("p (g e) -> p g e", g=NPAIR),
                            )
                            nc.vector.memset(vv[:, :, 128:130], 1.0)
                            for g in range(NPAIR):
                                nc.tensor.matmul(
                                    ctx_ps[g],
                                    lhsT=ek[:, g * 128 : (g + 1) * 128],
                                    rhs=vv[:, g, :],
                                    start=False,
                                    stop=(j == n_tb - 1),
                                )
                            last_vv = vv

                    # Dummy matmuls keep the PE busy across the vector-serial
                    # finalize below so the HAM activity monitor doesn't
                    # re-throttle the clock for pass B.  Reading the LAST vv
                    # tile pins their schedule to the end of pass A (operands
                    # with no late deps would let the scheduler hoist them
                    # into mid-pass A); emitting them inside this pool scope
                    # avoids a pool-close barrier in front of them.
                    vvf = last_vv.rearrange("p g e -> p (g e)")
                    kps = pq_pool.tile([128, 512], f32, name="pq")
                    for i in range(14):
                        nc.tensor.matmul(
                            kps, lhsT=vvf[:, 0:128], rhs=vvf[:, 0:512],
                            start=(i == 0), stop=(i == 13),
                        )

                # ---- finalize: normalize ctx into block-diagonal lhsT ----
                ctxbd = cpool.tile([128, NPAIR * 128], bf16, name="ctxbd", tag="ctxbd")
                nc.vector.memset(ctxbd, 0.0)
                for g in range(NPAIR):
                    ps = ctx_ps[g]
                    rk = small.tile([128, 1], f32, name="rk", tag="rk")
                    nc.vector.reciprocal(rk, ps[:, 128:129])
                    o = g * 128
                    nc.vector.tensor_scalar_mul(
                        ctxbd[0:64, o : o + 64], ps[0:64, 0:64], rk[0:64]
                    )
                    nc.vector.tensor_scalar_mul(
                        ctxbd[64:128, o + 64 : o + 128], ps[64:128, 64:128], rk[64:128]
                    )

            # ---- pass B: outT[e, t] = (ctxn^T expq)[e, t] per pair ----
            with tc.tile_pool(name="po", bufs=6, space="PSUM") as po_pool:
                for ci, (c0, cw) in enumerate(chunks):
                    for g in range(NPAIR):
                        po = po_pool.tile([128, cw], f32, name="po")
                        nc.tensor.matmul(
                            po,
                            lhsT=ctxbd[:, g * 128 : (g + 1) * 128],
                            rhs=expq_sb[:, g, c0 : c0 + cw],
                            start=True,
                            stop=True,
                        )
                        osb = osb_pool.tile([128, cw], bf16, name="osb")
                        if (ci + g) % 2 == 0:
                            nc.vector.tensor_copy(osb, po)
                        else:
                            nc.scalar.activation(osb, po, AF.Copy)
                        nc.sync.dma_start(out=outT[g, :, c0 : c0 + cw], in_=osb)

    nc.compile()
    return nc


def _host_inputs(x, w_qkv, mem_kv, mask, ntok=NTOK, mode=MODE):
    """Build the 8 per-core input maps on the host; returns (maps, idx list)."""
    import ml_dtypes

    fp8 = mode == "fp8"
    xnp = ml_dtypes.float8_e4m3 if fp8 else ml_dtypes.bfloat16
    ws = WS if fp8 else 1.0

    x = np.asarray(x, dtype=np.float32)
    w_qkv = np.asarray(w_qkv, dtype=np.float32)
    mem_kv = np.asarray(mem_kv, dtype=np.float32)
    mask = np.asarray(mask)

    nb = x.shape[0]
    n_tb = ntok // 128

    idxs, xTs, xTbs, biasms = [], [], [], []
    for b in range(nb):
        idx = np.nonzero(mask[b])[0]
        n = len(idx)
        assert n <= ntok, f"unmasked tokens {n} > capacity {ntok}"
        idxs.append(idx)
        xg = np.zeros((ntok, D_MODEL), np.float32)
        xg[:n] = x[b][idx]
        # chunk-major layout [128, sum_c(NDB*cw)]: element
        # [p, NDB*c0 + db*cw + t] = x[db*128+p, c0+t] -> each per-chunk DMA
        # reads one contiguous 4KB-per-partition block
        xc = np.empty((128, NDB * ntok), np.float32)
        for c0, cw in _chunks(ntok):
            blk = (
                xg[c0 : c0 + cw, :].T.reshape(NDB, 128, cw)
                .transpose(1, 0, 2)
                .reshape(128, NDB * cw)
            )
            xc[:, NDB * c0 : NDB * (c0 + cw)] = blk
        xTs.append(xc.astype(xnp))
        if fp8:
            xTbs.append(xc.astype(ml_dtypes.bfloat16))
        bm = np.zeros(ntok, np.float32)
        bm[n:] = -1e30
        biasms.append(np.ascontiguousarray(bm.reshape(n_tb, 128).T))

    # weights: [3072, 1024] -> per (half, proj): [128, NDB, ECOLS]
    w4 = w_qkv.reshape(N_HEADS, D_HEAD, 3, D_MODEL)
    wT = {}
    for half in (0, 1):
        h0 = half * HPC
        for ci, cn in ((0, "q"), (1, "k"), (2, "v")):
            wdt = ml_dtypes.bfloat16 if cn == "v" else xnp
            wsc = 1.0 if cn == "v" else ws
            w2 = w4[h0 : h0 + HPC, :, ci, :].reshape(ECOLS, D_MODEL) * wsc
            # [cols, d] -> [p, db, cols] with d = db*128 + p
            wT[(half, cn)] = np.ascontiguousarray(
                w2.T.reshape(NDB, 128, ECOLS).transpose(1, 0, 2)
            ).astype(wdt)

    in_maps = []
    for c in range(NCORES):
        b, half = divmod(c, 2)
        h0 = half * HPC
        mk = (
            mem_kv[0, h0 : h0 + HPC]
            .reshape(NPAIR, 2, NMEM, D_HEAD)
            .transpose(0, 2, 1, 3)
            .reshape(NPAIR, NMEM, 128)
        )
        mv = (
            mem_kv[1, h0 : h0 + HPC]
            .reshape(NPAIR, 2, NMEM, D_HEAD)
            .transpose(0, 2, 1, 3)
            .reshape(NPAIR, NMEM, 128)
        )
        mvp = np.ones((NPAIR, NMEM, 130), np.float32)
        mvp[:, :, :128] = mv
        im = {
            "xT": xTs[b],
            "wq": wT[(half, "q")],
            "wk": wT[(half, "k")],
            "wv": wT[(half, "v")],
            "mkp": np.exp(mk).astype(ml_dtypes.bfloat16),
            "mvp": mvp.astype(ml_dtypes.bfloat16),
            "biasm": biasms[b],
        }
        if fp8:
            im["xTb"] = xTbs[b]
        in_maps.append(im)
    return in_maps, idxs


def _assemble(results, idxs, nb=B, seqlen=L):
    """Divide num/den, transpose, scatter into the full (b, l, d) output."""
    out = np.zeros((nb, seqlen, D_MODEL), np.float32)
    for c in range(NCORES):
        b, half = divmod(c, 2)
        idx = idxs[b]
        n = len(idx)
        oT = np.asarray(results[c]["outT"]).astype(np.float32)  # [4, 128, ntok]
        eq = np.asarray(results[c]["expqd"]).astype(np.float32)  # [128, 4, ntok]
        # den[g, h, t] = sum_d expq[h*64+d, g, t]
        den = eq[:, :, :n].reshape(2, 64, NPAIR, n).sum(axis=1)  # [2, 4, n]
        num = oT[:, :, :n].reshape(NPAIR, 2, 64, n)
        y = num / den.transpose(1, 0, 2)[:, :, None, :]
        y = y.transpose(3, 0, 1, 2).reshape(n, ECOLS)
        out[b, idx, half * ECOLS : (half + 1) * ECOLS] = y
    return out


def _get_nc(ntok=NTOK, mode=MODE):
    key = (ntok, mode)
    if key not in _CACHE:
        _CACHE[key] = build_nc(ntok, mode)
    return _CACHE[key]


def kernel(x, w_qkv, mem_kv, mask):
    from concourse.bass_utils import run_bass_kernel_spmd

    mask = np.asarray(mask)
    ntok = NTOK
    max_n = int(mask.sum(axis=1).max())
    if max_n > ntok:  # safety net for unexpected mask densities
        ntok = -(-max_n // 128) * 128
    nc = _get_nc(ntok)
    in_maps, idxs = _host_inputs(x, w_qkv, mem_kv, mask, ntok=ntok)
    res = run_bass_kernel_spmd(nc, in_maps, core_ids=list(range(NCORES)))
    return _assemble(res.results, idxs, nb=x.shape[0], seqlen=x.shape[1])


# revision 54
# speedup vs baseline: 1.2835x; 1.0849x over previous
"""Linear-attention Trainium2 Bass kernel (optimized).

Reference computation (per batch b, head h):
    qkv = x @ W^T                         (t, 3072)
    q,k,v -> (h, t, 64)
    k masked rows -> -inf; prepend 4 mem-kv rows
    q = softmax(q * d^-0.5, axis=feature)
    k = softmax(k, axis=sequence)
    ctx = k^T v   (64x64);  out = q @ ctx;  out *= mask

Key optimizations over the naive mapping:
  * Masked tokens (~50%) contribute nothing to ctx (their k-softmax weight is
    0) and their output rows are zeroed; the host gathers only unmasked tokens
    and pads to NTOK=2304, nearly halving all projection work.
  * q/k projections run in fp8e4m3 with DoubleRow perf mode (256-row
    contraction per PE pass = 2x bf16 throughput); weights are pre-scaled by
    32 so fp8 keeps mantissa bits, and the 1/32 is folded into the exp
    epilogues.  v must stay bf16: softmax cancellation protects q/k from fp8
    noise but the v path has no such cancellation (fp8 anywhere in v costs
    ~2.5% output error).
  * k-softmax denominator = ones-column appended to v in the context matmul;
    padding rows are killed by a -1e30 exp bias.
  * Pass B emits the output TRANSPOSED ([head-col, token]) and exp(q) is
    DMAed out during pass A; the q-softmax denominator + divide happen on the
    host, so pass B is pure matmul + cast.
  * Dummy matmuls bridge PE-idle windows (startup DMA, ctx finalize) to keep
    the HAM clock-gate at full speed.

Sharding: 8 cores = (batch 0..3) x (head-half 0..1); no cross-core traffic.
"""

import numpy as np

D_MODEL = 1024
N_HEADS = 16
D_HEAD = 64
NMEM = 4
SCALE = D_HEAD ** -0.5
B = 4
L = 4096
NCORES = 8
HPC = 8            # heads per core
NPAIR = HPC // 2   # head-pairs per core
ECOLS = HPC * D_HEAD  # 512 output columns per core
NDB = D_MODEL // 128  # 8 contraction blocks

NTOK = 2176        # padded gathered-token capacity (>= max unmasked per batch;
                   # binom(4096,1/2) makes >2176 a ~4-sigma event, and kernel()
                   # falls back to a larger rebuild if an input ever exceeds it)
MODE = "fp8"       # "fp8": q/k in fp8 DoubleRow, v in bf16.  "bf16": all bf16.
WS = 32.0          # weight pre-scale for fp8 (folded back in epilogues)

_CACHE = {}


def _chunks(ntok):
    out = []
    c0 = 0
    while c0 < ntok:
        cw = min(512, ntok - c0)
        out.append((c0, cw))
        c0 += cw
    return out


def build_nc(ntok=NTOK, mode=MODE):
    """Build the per-core Bass program (identical across cores; data differs)."""
    import concourse.tile as tile
    from concourse import bacc, mybir

    f32 = mybir.dt.float32
    bf16 = mybir.dt.bfloat16
    AF = mybir.ActivationFunctionType
    DR = mybir.MatmulPerfMode.DoubleRow

    fp8 = mode == "fp8"
    xdt = mybir.dt.float8e4 if fp8 else bf16
    ws = WS if fp8 else 1.0

    n_tb = ntok // 128
    chunks = _chunks(ntok)

    nc = bacc.Bacc("TRN2", target_bir_lowering=False, debug=False)

    # x streams are laid out CHUNK-MAJOR on the host ([p, chunk|(db t)]) so each
    # per-chunk DMA is one contiguous 4KB-per-partition block (the naive
    # [d_model, t] slice produces 512B blocks that run at ~1/3 DMA bandwidth)
    xT = nc.dram_tensor("xT", (128, NDB * ntok), xdt, kind="ExternalInput").ap()
    if fp8:  # v path stays bf16 (fp8 wv/x injects ~2.5% output error)
        xTb = nc.dram_tensor("xTb", (128, NDB * ntok), bf16, kind="ExternalInput").ap()
    wq = nc.dram_tensor("wq", (128, NDB, ECOLS), xdt, kind="ExternalInput").ap()
    wk = nc.dram_tensor("wk", (128, NDB, ECOLS), xdt, kind="ExternalInput").ap()
    wv = nc.dram_tensor("wv", (128, NDB, ECOLS), bf16, kind="ExternalInput").ap()
    mkp = nc.dram_tensor("mkp", (NPAIR, NMEM, 128), bf16, kind="ExternalInput").ap()
    mvp = nc.dram_tensor("mvp", (NPAIR, NMEM, 130), bf16, kind="ExternalInput").ap()
    biasm = nc.dram_tensor("biasm", (128, n_tb), f32, kind="ExternalInput").ap()
    # packed pass-B output: column [4*c0 + g*cw + t] for chunk (c0, cw) --
    # pairs sit side-by-side along the free dim so each chunk needs ONE
    # contiguous 2D DMA (per-pair DMAs cost 0.6us of desc-gen each and were
    # pacing pass B; a pair-major batched DMA costs ~4us of desc-gen)
    outT = nc.dram_tensor("outT", (128, NPAIR * ntok), bf16, kind="ExternalOutput").ap()
    expqd = nc.dram_tensor("expqd", (128, NPAIR, ntok), bf16, kind="ExternalOutput").ap()

    with tile.TileContext(nc) as tc:
        with (
            tc.tile_pool(name="const", bufs=1) as cpool,
            tc.tile_pool(name="big", bufs=1) as bigpool,
            tc.tile_pool(name="small", bufs=8) as small,
            tc.tile_pool(name="xt", bufs=2) as xt_pool,
            tc.tile_pool(name="xtb", bufs=2) as xtb_pool,
            tc.tile_pool(name="ek", bufs=5) as ek_pool,
            tc.tile_pool(name="vv", bufs=3) as vv_pool,
            tc.tile_pool(name="osb", bufs=4) as osb_pool,
        ):
            # ---- PE warm-up: dense dummy matmuls while DMAs land ----
            with (
                tc.tile_pool(name="warm", bufs=1) as warm_pool,
                tc.tile_pool(name="warmps", bufs=1, space="PSUM") as warmps_pool,
            ):
                # sized to span the initial weight/x DMA window (~5-6us at the
                # cold 1.2 GHz clock) so the PE never goes idle long enough
                # for the HAM monitor to re-throttle before pass A starts
                wrm = warm_pool.tile([128, 512], bf16, name="wrm", tag="wrm")
                nc.vector.memset(wrm, 0.0)
                wps = warmps_pool.tile([128, 512], f32, name="wps", tag="wps")
                for i in range(9):
                    nc.tensor.matmul(
                        wps, lhsT=wrm[:, 0:128], rhs=wrm,
                        start=(i == 0), stop=(i == 8),
                    )

            # ---- constants / weights ----
            w_sbs = []
            for nm, w_dram, wdt in (("wq", wq, xdt), ("wk", wk, xdt), ("wv", wv, bf16)):
                w_sb = cpool.tile([128, NDB, ECOLS], wdt, name=f"{nm}_sb", tag=f"{nm}_sb")
                nc.sync.dma_start(out=w_sb, in_=w_dram)
                w_sbs.append(w_sb)
            wq_sb, wk_sb, wv_sb = w_sbs

            mk_sb = cpool.tile([NMEM, NPAIR * 128], bf16, name="mk_sb", tag="mk_sb")
            nc.sync.dma_start(
                out=mk_sb.rearrange("n (g d) -> n g d", g=NPAIR),
                in_=mkp.rearrange("g n d -> n g d"),
            )
            mv_sb = cpool.tile([NMEM, NPAIR * 130], bf16, name="mv_sb", tag="mv_sb")
            nc.sync.dma_start(
                out=mv_sb.rearrange("n (g e) -> n g e", g=NPAIR),
                in_=mvp.rearrange("g n e -> n g e"),
            )

            biasm_sb = cpool.tile([128, n_tb], f32, name="biasm_sb", tag="biasm_sb")
            nc.sync.dma_start(out=biasm_sb, in_=biasm)

            # exp(q * scale) for the whole batch, kept resident: [128, pair, t]
            expq_sb = bigpool.tile([128, NPAIR, ntok], bf16, name="expq_sb", tag="expq_sb")

            with tc.tile_pool(name="ctxps", bufs=1, space="PSUM") as ctx_pool:
                # persistent context accumulators (one psum bank per pair)
                ctx_ps = [
                    ctx_pool.tile([128, 130], f32, name=f"ctx_ps{i}", tag=f"ctx{i}")
                    for i in range(NPAIR)
                ]

                # ---- mem-kv rows initialize the context accumulation ----
                for g in range(NPAIR):
                    nc.tensor.matmul(
                        ctx_ps[g],
                        lhsT=mk_sb[:, g * 128 : (g + 1) * 128],
                        rhs=mv_sb[:, g * 130 : (g + 1) * 130],
                        start=True,
                        stop=False,
                    )

                # ---- pass A: projection + exp + context accumulation ----
                with (
                    tc.tile_pool(name="pq", bufs=2, space="PSUM") as pq_pool,
                    tc.tile_pool(name="pk", bufs=1, space="PSUM") as pk_pool,
                    tc.tile_pool(name="pv", bufs=1, space="PSUM") as pv_pool,
                ):
                    for ci, (c0, cw) in enumerate(chunks):
                        # x streams ride the Activation HWDGE queue so they
                        # don't serialize behind weight/output DMAs on SP.
                        xt = xt_pool.tile([128, NDB, cw], xdt, name="xt")
                        nc.scalar.dma_start(
                            out=xt,
                            in_=xT[:, NDB * c0 : NDB * (c0 + cw)].rearrange(
                                "p (db t) -> p db t", db=NDB
                            ),
                        )
                        if fp8:
                            xtb = xtb_pool.tile([128, NDB, cw], bf16, name="xtb")
                            nc.scalar.dma_start(
                                out=xtb,
                                in_=xTb[:, NDB * c0 : NDB * (c0 + cw)].rearrange(
                                    "p (db t) -> p db t", db=NDB
                                ),
                            )
                        else:
                            xtb = xt

                        # qT projection per pair: psum [128 = pair-dheads, cw]
                        for g in range(NPAIR):
                            pq = pq_pool.tile([128, cw], f32, name="pq")
                            if fp8:
                                for s in range(NDB // 2):
                                    nc.tensor.matmul(
                                        pq,
                                        lhsT=wq_sb[:, 2 * s : 2 * s + 2, g * 128 : (g + 1) * 128],
                                        rhs=xt[:, 2 * s : 2 * s + 2, :],
                                        start=(s == 0),
                                        stop=(s == NDB // 2 - 1),
                                        perf_mode=DR,
                                    )
                            else:
                                for db in range(NDB):
                                    nc.tensor.matmul(
                                        pq,
                                        lhsT=wq_sb[:, db, g * 128 : (g + 1) * 128],
                                        rhs=xt[:, db, :],
                                        start=(db == 0),
                                        stop=(db == NDB - 1),
                                    )
                            nc.scalar.activation(
                                expq_sb[:, g, c0 : c0 + cw], pq, AF.Exp, scale=SCALE / ws
                            )
                        # ship exp(q) to the host (q-softmax denominator +
                        # divide happen host-side; pass B stays matmul-dense)
                        nc.sync.dma_start(
                            out=expqd[:, :, c0 : c0 + cw],
                            in_=expq_sb[:, :, c0 : c0 + cw],
                        )

                        # k/v projection + exp(k)+padkill + context, per t-block
                        for tbi in range(cw // 128):
                            j = c0 // 128 + tbi
                            t0, t1 = tbi * 128, (tbi + 1) * 128
                            pk = pk_pool.tile([128, ECOLS], f32, name="pk")
                            pv = pv_pool.tile([128, ECOLS], f32, name="pv")
                            if fp8:
                                for s in range(NDB // 2):
                                    nc.tensor.matmul(
                                        pk,
                                        lhsT=xt[:, 2 * s : 2 * s + 2, t0:t1],
                                        rhs=wk_sb[:, 2 * s : 2 * s + 2, :],
                                        start=(s == 0),
                                        stop=(s == NDB // 2 - 1),
                                        perf_mode=DR,
                                    )
                            else:
                                for db in range(NDB):
                                    nc.tensor.matmul(
                                        pk,
                                        lhsT=xt[:, db, t0:t1],
                                        rhs=wk_sb[:, db, :],
                                        start=(db == 0),
                                        stop=(db == NDB - 1),
                                    )
                            for db in range(NDB):
                                nc.tensor.matmul(
                                    pv,
                                    lhsT=xtb[:, db, t0:t1],
                                    rhs=wv_sb[:, db, :],
                                    start=(db == 0),
                                    stop=(db == NDB - 1),
                                )
                            ek = ek_pool.tile([128, ECOLS], bf16, name="ek")
                            nc.scalar.activation(
                                ek, pk, AF.Exp, bias=biasm_sb[:, j : j + 1], scale=1.0 / ws
                            )
                            vv = vv_pool.tile([128, NPAIR, 130], bf16, name="vv")
                            nc.vector.tensor_copy(
                                vv[:, :, 0:128],
                                pv.rearrange("p (g e) -> p g e", g=NPAIR),
                            )
                            nc.vector.memset(vv[:, :, 128:130], 1.0)
                            for g in range(NPAIR):
                                nc.tensor.matmul(
                                    ctx_ps[g],
                                    lhsT=ek[:, g * 128 : (g + 1) * 128],
                                    rhs=vv[:, g, :],
                                    start=False,
                                    stop=(j == n_tb - 1),
                                )
                            last_vv = vv

                    # Dummy matmuls keep the PE busy across the vector-serial
                    # finalize below so the HAM activity monitor doesn't
                    # re-throttle the clock for pass B.  Reading the LAST vv
                    # tile pins their schedule to the end of pass A (operands
                    # with no late deps would let the scheduler hoist them
                    # into mid-pass A); emitting them inside this pool scope
                    # avoids a pool-close barrier in front of them.
                    vvf = last_vv.rearrange("p g e -> p (g e)")
                    kps = pq_pool.tile([128, 512], f32, name="pq")
                    for i in range(14):
                        nc.tensor.matmul(
                            kps, lhsT=vvf[:, 0:128], rhs=vvf[:, 0:512],
                            start=(i == 0), stop=(i == 13),
                        )

                # ---- finalize: normalize ctx into block-diagonal lhsT ----
                ctxbd = cpool.tile([128, NPAIR * 128], bf16, name="ctxbd", tag="ctxbd")
                nc.vector.memset(ctxbd, 0.0)
                for g in range(NPAIR):
                    ps = ctx_ps[g]
                    rk = small.tile([128, 1], f32, name="rk", tag="rk")
                    nc.vector.reciprocal(rk, ps[:, 128:129])
                    o = g * 128
                    nc.vector.tensor_scalar_mul(
                        ctxbd[0:64, o : o + 64], ps[0:64, 0:64], rk[0:64]
                    )
                    nc.vector.tensor_scalar_mul(
                        ctxbd[64:128, o + 64 : o + 128], ps[64:128, 64:128], rk[64:128]
                    )

            # ---- pass B: outT[e, t] = (ctxn^T expq)[e, t] per pair ----
            with tc.tile_pool(name="po", bufs=6, space="PSUM") as po_pool:
                for ci, (c0, cw) in enumerate(chunks):
                    osb = osb_pool.tile([128, NPAIR * cw], bf16, name="osb")
                    for g in range(NPAIR):
                        po = po_pool.tile([128, cw], f32, name="po")
                        nc.tensor.matmul(
                            po,
                            lhsT=ctxbd[:, g * 128 : (g + 1) * 128],
                            rhs=expq_sb[:, g, c0 : c0 + cw],
                            start=True,
                            stop=True,
                        )
                        if (ci + g) % 2 == 0:
                            nc.vector.tensor_copy(osb[:, g * cw : (g + 1) * cw], po)
                        else:
                            nc.scalar.activation(osb[:, g * cw : (g + 1) * cw], po, AF.Copy)
                    nc.sync.dma_start(
                        out=outT[:, NPAIR * c0 : NPAIR * (c0 + cw)], in_=osb
                    )

    nc.compile()
    return nc


def _host_inputs(x, w_qkv, mem_kv, mask, ntok=NTOK, mode=MODE):
    """Build the 8 per-core input maps on the host; returns (maps, idx list)."""
    import ml_dtypes

    fp8 = mode == "fp8"
    xnp = ml_dtypes.float8_e4m3 if fp8 else ml_dtypes.bfloat16
    ws = WS if fp8 else 1.0

    x = np.asarray(x, dtype=np.float32)
    w_qkv = np.asarray(w_qkv, dtype=np.float32)
    mem_kv = np.asarray(mem_kv, dtype=np.float32)
    mask = np.asarray(mask)

    nb = x.shape[0]
    n_tb = ntok // 128

    idxs, xTs, xTbs, biasms = [], [], [], []
    for b in range(nb):
        idx = np.nonzero(mask[b])[0]
        n = len(idx)
        assert n <= ntok, f"unmasked tokens {n} > capacity {ntok}"
        idxs.append(idx)
        xg = np.zeros((ntok, D_MODEL), np.float32)
        xg[:n] = x[b][idx]
        # chunk-major layout [128, sum_c(NDB*cw)]: element
        # [p, NDB*c0 + db*cw + t] = x[db*128+p, c0+t] -> each per-chunk DMA
        # reads one contiguous 4KB-per-partition block
        xc = np.empty((128, NDB * ntok), np.float32)
        for c0, cw in _chunks(ntok):
            blk = (
                xg[c0 : c0 + cw, :].T.reshape(NDB, 128, cw)
                .transpose(1, 0, 2)
                .reshape(128, NDB * cw)
            )
            xc[:, NDB * c0 : NDB * (c0 + cw)] = blk
        xTs.append(xc.astype(xnp))
        if fp8:
            xTbs.append(xc.astype(ml_dtypes.bfloat16))
        bm = np.zeros(ntok, np.float32)
        bm[n:] = -1e30
        biasms.append(np.ascontiguousarray(bm.reshape(n_tb, 128).T))

    # weights: [3072, 1024] -> per (half, proj): [128, NDB, ECOLS]
    w4 = w_qkv.reshape(N_HEADS, D_HEAD, 3, D_MODEL)
    wT = {}
    for half in (0, 1):
        h0 = half * HPC
        for ci, cn in ((0, "q"), (1, "k"), (2, "v")):
            wdt = ml_dtypes.bfloat16 if cn == "v" else xnp
            wsc = 1.0 if cn == "v" else ws
            w2 = w4[h0 : h0 + HPC, :, ci, :].reshape(ECOLS, D_MODEL) * wsc
            # [cols, d] -> [p, db, cols] with d = db*128 + p
            wT[(half, cn)] = np.ascontiguousarray(
                w2.T.reshape(NDB, 128, ECOLS).transpose(1, 0, 2)
            ).astype(wdt)

    in_maps = []
    for c in range(NCORES):
        b, half = divmod(c, 2)
        h0 = half * HPC
        mk = (
            mem_kv[0, h0 : h0 + HPC]
            .reshape(NPAIR, 2, NMEM, D_HEAD)
            .transpose(0, 2, 1, 3)
            .reshape(NPAIR, NMEM, 128)
        )
        mv = (
            mem_kv[1, h0 : h0 + HPC]
            .reshape(NPAIR, 2, NMEM, D_HEAD)
            .transpose(0, 2, 1, 3)
            .reshape(NPAIR, NMEM, 128)
        )
        mvp = np.ones((NPAIR, NMEM, 130), np.float32)
        mvp[:, :, :128] = mv
        im = {
            "xT": xTs[b],
            "wq": wT[(half, "q")],
            "wk": wT[(half, "k")],
            "wv": wT[(half, "v")],
            "mkp": np.exp(mk).astype(ml_dtypes.bfloat16),
            "mvp": mvp.astype(ml_dtypes.bfloat16),
            "biasm": biasms[b],
        }
        if fp8:
            im["xTb"] = xTbs[b]
        in_maps.append(im)
    return in_maps, idxs


def _assemble(results, idxs, nb=B, seqlen=L):
    """Divide num/den, transpose, scatter into the full (b, l, d) output."""
    out = np.zeros((nb, seqlen, D_MODEL), np.float32)
    for c in range(NCORES):
        b, half = divmod(c, 2)
        idx = idxs[b]
        n = len(idx)
        oTp = np.asarray(results[c]["outT"]).astype(np.float32)  # [128, 4*ntok] packed
        ntok = oTp.shape[1] // NPAIR
        oT = np.empty((NPAIR, 128, ntok), np.float32)
        for c0, cw in _chunks(ntok):
            blk = oTp[:, NPAIR * c0 : NPAIR * (c0 + cw)].reshape(128, NPAIR, cw)
            oT[:, :, c0 : c0 + cw] = blk.transpose(1, 0, 2)
        eq = np.asarray(results[c]["expqd"]).astype(np.float32)  # [128, 4, ntok]
        # den[g, h, t] = sum_d expq[h*64+d, g, t]
        den = eq[:, :, :n].reshape(2, 64, NPAIR, n).sum(axis=1)  # [2, 4, n]
        num = oT[:, :, :n].reshape(NPAIR, 2, 64, n)
        y = num / den.transpose(1, 0, 2)[:, :, None, :]
        y = y.transpose(3, 0, 1, 2).reshape(n, ECOLS)
        out[b, idx, half * ECOLS : (half + 1) * ECOLS] = y
    return out


def _get_nc(ntok=NTOK, mode=MODE):
    key = (ntok, mode)
    if key not in _CACHE:
        _CACHE[key] = build_nc(ntok, mode)
    return _CACHE[key]


def kernel(x, w_qkv, mem_kv, mask):
    from concourse.bass_utils import run_bass_kernel_spmd

    mask = np.asarray(mask)
    ntok = NTOK
    max_n = int(mask.sum(axis=1).max())
    if max_n > ntok:  # safety net for unexpected mask densities
        ntok = -(-max_n // 128) * 128
    nc = _get_nc(ntok)
    in_maps, idxs = _host_inputs(x, w_qkv, mem_kv, mask, ntok=ntok)
    res = run_bass_kernel_spmd(nc, in_maps, core_ids=list(range(NCORES)))
    return _assemble(res.results, idxs, nb=x.shape[0], seqlen=x.shape[1])


# revision 56
# speedup vs baseline: 1.2907x; 1.0056x over previous
"""Linear-attention Trainium2 Bass kernel (optimized).

Reference computation (per batch b, head h):
    qkv = x @ W^T                         (t, 3072)
    q,k,v -> (h, t, 64)
    k masked rows -> -inf; prepend 4 mem-kv rows
    q = softmax(q * d^-0.5, axis=feature)
    k = softmax(k, axis=sequence)
    ctx = k^T v   (64x64);  out = q @ ctx;  out *= mask

Key optimizations over the naive mapping:
  * Masked tokens (~50%) contribute nothing to ctx (their k-softmax weight is
    0) and their output rows are zeroed; the host gathers only unmasked tokens
    and pads to NTOK=2304, nearly halving all projection work.
  * q/k projections run in fp8e4m3 with DoubleRow perf mode (256-row
    contraction per PE pass = 2x bf16 throughput); weights are pre-scaled by
    32 so fp8 keeps mantissa bits, and the 1/32 is folded into the exp
    epilogues.  v must stay bf16: softmax cancellation protects q/k from fp8
    noise but the v path has no such cancellation (fp8 anywhere in v costs
    ~2.5% output error).
  * k-softmax denominator = ones-column appended to v in the context matmul;
    padding rows are killed by a -1e30 exp bias.
  * Pass B emits the output TRANSPOSED ([head-col, token]) and exp(q) is
    DMAed out during pass A; the q-softmax denominator + divide happen on the
    host, so pass B is pure matmul + cast.
  * Dummy matmuls bridge PE-idle windows (startup DMA, ctx finalize) to keep
    the HAM clock-gate at full speed.

Sharding: 8 cores = (batch 0..3) x (head-half 0..1); no cross-core traffic.
"""

import numpy as np

D_MODEL = 1024
N_HEADS = 16
D_HEAD = 64
NMEM = 4
SCALE = D_HEAD ** -0.5
B = 4
L = 4096
NCORES = 8
HPC = 8            # heads per core
NPAIR = HPC // 2   # head-pairs per core
ECOLS = HPC * D_HEAD  # 512 output columns per core
NDB = D_MODEL // 128  # 8 contraction blocks

NTOK = 2176        # padded gathered-token capacity (>= max unmasked per batch;
                   # binom(4096,1/2) makes >2176 a ~4-sigma event, and kernel()
                   # falls back to a larger rebuild if an input ever exceeds it)
MODE = "fp8"       # "fp8": q/k in fp8 DoubleRow, v in bf16.  "bf16": all bf16.
WS = 32.0          # weight pre-scale for fp8 (folded back in epilogues)

_CACHE = {}


def _chunks(ntok):
    out = []
    c0 = 0
    while c0 < ntok:
        cw = min(512, ntok - c0)
        out.append((c0, cw))
        c0 += cw
    return out


def build_nc(ntok=NTOK, mode=MODE):
    """Build the per-core Bass program (identical across cores; data differs)."""
    import concourse.tile as tile
    from concourse import bacc, mybir

    f32 = mybir.dt.float32
    bf16 = mybir.dt.bfloat16
    AF = mybir.ActivationFunctionType
    DR = mybir.MatmulPerfMode.DoubleRow

    fp8 = mode == "fp8"
    xdt = mybir.dt.float8e4 if fp8 else bf16
    ws = WS if fp8 else 1.0

    n_tb = ntok // 128
    chunks = _chunks(ntok)

    nc = bacc.Bacc("TRN2", target_bir_lowering=False, debug=False)

    # x streams are laid out CHUNK-MAJOR on the host ([p, chunk|(db t)]) so each
    # per-chunk DMA is one contiguous 4KB-per-partition block (the naive
    # [d_model, t] slice produces 512B blocks that run at ~1/3 DMA bandwidth)
    xT = nc.dram_tensor("xT", (128, NDB * ntok), xdt, kind="ExternalInput").ap()
    if fp8:  # v path stays bf16 (fp8 wv/x injects ~2.5% output error)
        xTb = nc.dram_tensor("xTb", (128, NDB * ntok), bf16, kind="ExternalInput").ap()
    wq = nc.dram_tensor("wq", (128, NDB, ECOLS), xdt, kind="ExternalInput").ap()
    wk = nc.dram_tensor("wk", (128, NDB, ECOLS), xdt, kind="ExternalInput").ap()
    wv = nc.dram_tensor("wv", (128, NDB, ECOLS), bf16, kind="ExternalInput").ap()
    mkp = nc.dram_tensor("mkp", (NPAIR, NMEM, 128), bf16, kind="ExternalInput").ap()
    mvp = nc.dram_tensor("mvp", (NPAIR, NMEM, 130), bf16, kind="ExternalInput").ap()
    biasm = nc.dram_tensor("biasm", (128, n_tb), f32, kind="ExternalInput").ap()
    # packed pass-B output: column [4*c0 + g*cw + t] for chunk (c0, cw) --
    # pairs sit side-by-side along the free dim so each chunk needs ONE
    # contiguous 2D DMA (per-pair DMAs cost 0.6us of desc-gen each and were
    # pacing pass B; a pair-major batched DMA costs ~4us of desc-gen)
    outT = nc.dram_tensor("outT", (128, NPAIR * ntok), bf16, kind="ExternalOutput").ap()
    expqd = nc.dram_tensor("expqd", (128, NPAIR, ntok), bf16, kind="ExternalOutput").ap()

    with tile.TileContext(nc) as tc:
        with (
            tc.tile_pool(name="const", bufs=1) as cpool,
            tc.tile_pool(name="big", bufs=1) as bigpool,
            tc.tile_pool(name="small", bufs=8) as small,
            tc.tile_pool(name="xt", bufs=2) as xt_pool,
            tc.tile_pool(name="xtb", bufs=2) as xtb_pool,
            tc.tile_pool(name="ek", bufs=5) as ek_pool,
            tc.tile_pool(name="vv", bufs=3) as vv_pool,
            tc.tile_pool(name="osb", bufs=4) as osb_pool,
        ):
            # ---- PE warm-up: dense dummy matmuls while DMAs land ----
            with (
                tc.tile_pool(name="warm", bufs=1) as warm_pool,
                tc.tile_pool(name="warmps", bufs=1, space="PSUM") as warmps_pool,
            ):
                # sized to span the initial weight/x DMA window (~5-6us at the
                # cold 1.2 GHz clock) so the PE never goes idle long enough
                # for the HAM monitor to re-throttle before pass A starts
                wrm = warm_pool.tile([128, 512], bf16, name="wrm", tag="wrm")
                nc.vector.memset(wrm, 0.0)
                wps = warmps_pool.tile([128, 512], f32, name="wps", tag="wps")
                for i in range(7):
                    nc.tensor.matmul(
                        wps, lhsT=wrm[:, 0:128], rhs=wrm,
                        start=(i == 0), stop=(i == 6),
                    )

            # ---- constants / weights ----
            w_sbs = []
            for nm, w_dram, wdt in (("wq", wq, xdt), ("wk", wk, xdt), ("wv", wv, bf16)):
                w_sb = cpool.tile([128, NDB, ECOLS], wdt, name=f"{nm}_sb", tag=f"{nm}_sb")
                nc.sync.dma_start(out=w_sb, in_=w_dram)
                w_sbs.append(w_sb)
            wq_sb, wk_sb, wv_sb = w_sbs

            mk_sb = cpool.tile([NMEM, NPAIR * 128], bf16, name="mk_sb", tag="mk_sb")
            nc.sync.dma_start(
                out=mk_sb.rearrange("n (g d) -> n g d", g=NPAIR),
                in_=mkp.rearrange("g n d -> n g d"),
            )
            mv_sb = cpool.tile([NMEM, NPAIR * 130], bf16, name="mv_sb", tag="mv_sb")
            nc.sync.dma_start(
                out=mv_sb.rearrange("n (g e) -> n g e", g=NPAIR),
                in_=mvp.rearrange("g n e -> n g e"),
            )

            biasm_sb = cpool.tile([128, n_tb], f32, name="biasm_sb", tag="biasm_sb")
            nc.sync.dma_start(out=biasm_sb, in_=biasm)

            # exp(q * scale) for the whole batch, kept resident: [128, pair, t]
            expq_sb = bigpool.tile([128, NPAIR, ntok], bf16, name="expq_sb", tag="expq_sb")

            with tc.tile_pool(name="ctxps", bufs=1, space="PSUM") as ctx_pool:
                # persistent context accumulators (one psum bank per pair)
                ctx_ps = [
                    ctx_pool.tile([128, 130], f32, name=f"ctx_ps{i}", tag=f"ctx{i}")
                    for i in range(NPAIR)
                ]

                # ---- mem-kv rows initialize the context accumulation ----
                for g in range(NPAIR):
                    nc.tensor.matmul(
                        ctx_ps[g],
                        lhsT=mk_sb[:, g * 128 : (g + 1) * 128],
                        rhs=mv_sb[:, g * 130 : (g + 1) * 130],
                        start=True,
                        stop=False,
                    )

                # ---- pass A: projection + exp + context accumulation ----
                with (
                    tc.tile_pool(name="pq", bufs=2, space="PSUM") as pq_pool,
                    tc.tile_pool(name="pk", bufs=1, space="PSUM") as pk_pool,
                    tc.tile_pool(name="pv", bufs=1, space="PSUM") as pv_pool,
                ):
                    for ci, (c0, cw) in enumerate(chunks):
                        # x streams ride the Activation HWDGE queue so they
                        # don't serialize behind weight/output DMAs on SP.
                        xt = xt_pool.tile([128, NDB, cw], xdt, name="xt")
                        nc.scalar.dma_start(
                            out=xt,
                            in_=xT[:, NDB * c0 : NDB * (c0 + cw)].rearrange(
                                "p (db t) -> p db t", db=NDB
                            ),
                        )
                        if fp8:
                            xtb = xtb_pool.tile([128, NDB, cw], bf16, name="xtb")
                            nc.scalar.dma_start(
                                out=xtb,
                                in_=xTb[:, NDB * c0 : NDB * (c0 + cw)].rearrange(
                                    "p (db t) -> p db t", db=NDB
                                ),
                            )
                        else:
                            xtb = xt

                        # qT projection per pair: psum [128 = pair-dheads, cw]
                        for g in range(NPAIR):
                            pq = pq_pool.tile([128, cw], f32, name="pq")
                            if fp8:
                                for s in range(NDB // 2):
                                    nc.tensor.matmul(
                                        pq,
                                        lhsT=wq_sb[:, 2 * s : 2 * s + 2, g * 128 : (g + 1) * 128],
                                        rhs=xt[:, 2 * s : 2 * s + 2, :],
                                        start=(s == 0),
                                        stop=(s == NDB // 2 - 1),
                                        perf_mode=DR,
                                    )
                            else:
                                for db in range(NDB):
                                    nc.tensor.matmul(
                                        pq,
                                        lhsT=wq_sb[:, db, g * 128 : (g + 1) * 128],
                                        rhs=xt[:, db, :],
                                        start=(db == 0),
                                        stop=(db == NDB - 1),
                                    )
                            nc.scalar.activation(
                                expq_sb[:, g, c0 : c0 + cw], pq, AF.Exp, scale=SCALE / ws
                            )
                        # ship exp(q) to the host (q-softmax denominator +
                        # divide happen host-side; pass B stays matmul-dense)
                        nc.sync.dma_start(
                            out=expqd[:, :, c0 : c0 + cw],
                            in_=expq_sb[:, :, c0 : c0 + cw],
                        )

                        # k/v projection + exp(k)+padkill + context, per t-block
                        for tbi in range(cw // 128):
                            j = c0 // 128 + tbi
                            t0, t1 = tbi * 128, (tbi + 1) * 128
                            pk = pk_pool.tile([128, ECOLS], f32, name="pk")
                            pv = pv_pool.tile([128, ECOLS], f32, name="pv")
                            if fp8:
                                for s in range(NDB // 2):
                                    nc.tensor.matmul(
                                        pk,
                                        lhsT=xt[:, 2 * s : 2 * s + 2, t0:t1],
                                        rhs=wk_sb[:, 2 * s : 2 * s + 2, :],
                                        start=(s == 0),
                                        stop=(s == NDB // 2 - 1),
                                        perf_mode=DR,
                                    )
                            else:
                                for db in range(NDB):
                                    nc.tensor.matmul(
                                        pk,
                                        lhsT=xt[:, db, t0:t1],
                                        rhs=wk_sb[:, db, :],
                                        start=(db == 0),
                                        stop=(db == NDB - 1),
                                    )
                            for db in range(NDB):
                                nc.tensor.matmul(
                                    pv,
                                    lhsT=xtb[:, db, t0:t1],
                                    rhs=wv_sb[:, db, :],
                                    start=(db == 0),
                                    stop=(db == NDB - 1),
                                )
                            ek = ek_pool.tile([128, ECOLS], bf16, name="ek")
                            nc.scalar.activation(
                                ek, pk, AF.Exp, bias=biasm_sb[:, j : j + 1], scale=1.0 / ws
                            )
                            vv = vv_pool.tile([128, NPAIR, 130], bf16, name="vv")
                            nc.vector.tensor_copy(
                                vv[:, :, 0:128],
                                pv.rearrange("p (g e) -> p g e", g=NPAIR),
                            )
                            nc.vector.memset(vv[:, :, 128:130], 1.0)
                            for g in range(NPAIR):
                                nc.tensor.matmul(
                                    ctx_ps[g],
                                    lhsT=ek[:, g * 128 : (g + 1) * 128],
                                    rhs=vv[:, g, :],
                                    start=False,
                                    stop=(j == n_tb - 1),
                                )
                            last_vv = vv

                    # Dummy matmuls keep the PE busy across the vector-serial
                    # finalize below so the HAM activity monitor doesn't
                    # re-throttle the clock for pass B.  Reading the LAST vv
                    # tile pins their schedule to the end of pass A (operands
                    # with no late deps would let the scheduler hoist them
                    # into mid-pass A); emitting them inside this pool scope
                    # avoids a pool-close barrier in front of them.
                    vvf = last_vv.rearrange("p g e -> p (g e)")
                    kps = pq_pool.tile([128, 512], f32, name="pq")
                    for i in range(12):
                        nc.tensor.matmul(
                            kps, lhsT=vvf[:, 0:128], rhs=vvf[:, 0:512],
                            start=(i == 0), stop=(i == 11),
                        )

                # ---- finalize: normalize ctx into block-diagonal lhsT ----
                ctxbd = cpool.tile([128, NPAIR * 128], bf16, name="ctxbd", tag="ctxbd")
                nc.vector.memset(ctxbd, 0.0)
                for g in range(NPAIR):
                    ps = ctx_ps[g]
                    rk = small.tile([128, 1], f32, name="rk", tag="rk")
                    nc.vector.reciprocal(rk, ps[:, 128:129])
                    o = g * 128
                    nc.vector.tensor_scalar_mul(
                        ctxbd[0:64, o : o + 64], ps[0:64, 0:64], rk[0:64]
                    )
                    nc.vector.tensor_scalar_mul(
                        ctxbd[64:128, o + 64 : o + 128], ps[64:128, 64:128], rk[64:128]
                    )

            # ---- pass B: outT[e, t] = (ctxn^T expq)[e, t] per pair ----
            with tc.tile_pool(name="po", bufs=6, space="PSUM") as po_pool:
                for ci, (c0, cw) in enumerate(chunks):
                    osb = osb_pool.tile([128, NPAIR * cw], bf16, name="osb")
                    for g in range(NPAIR):
                        po = po_pool.tile([128, cw], f32, name="po")
                        nc.tensor.matmul(
                            po,
                            lhsT=ctxbd[:, g * 128 : (g + 1) * 128],
                            rhs=expq_sb[:, g, c0 : c0 + cw],
                            start=True,
                            stop=True,
                        )
                        if (ci + g) % 2 == 0:
                            nc.vector.tensor_copy(osb[:, g * cw : (g + 1) * cw], po)
                        else:
                            nc.scalar.activation(osb[:, g * cw : (g + 1) * cw], po, AF.Copy)
                    nc.sync.dma_start(
                        out=outT[:, NPAIR * c0 : NPAIR * (c0 + cw)], in_=osb
                    )

    nc.compile()
    return nc


def _host_inputs(x, w_qkv, mem_kv, mask, ntok=NTOK, mode=MODE):
    """Build the 8 per-core input maps on the host; returns (maps, idx list)."""
    import ml_dtypes

    fp8 = mode == "fp8"
    xnp = ml_dtypes.float8_e4m3 if fp8 else ml_dtypes.bfloat16
    ws = WS if fp8 else 1.0

    x = np.asarray(x, dtype=np.float32)
    w_qkv = np.asarray(w_qkv, dtype=np.float32)
    mem_kv = np.asarray(mem_kv, dtype=np.float32)
    mask = np.asarray(mask)

    nb = x.shape[0]
    n_tb = ntok // 128

    idxs, xTs, xTbs, biasms = [], [], [], []
    for b in range(nb):
        idx = np.nonzero(mask[b])[0]
        n = len(idx)
        assert n <= ntok, f"unmasked tokens {n} > capacity {ntok}"
        idxs.append(idx)
        xg = np.zeros((ntok, D_MODEL), np.float32)
        xg[:n] = x[b][idx]
        # chunk-major layout [128, sum_c(NDB*cw)]: element
        # [p, NDB*c0 + db*cw + t] = x[db*128+p, c0+t] -> each per-chunk DMA
        # reads one contiguous 4KB-per-partition block
        xc = np.empty((128, NDB * ntok), np.float32)
        for c0, cw in _chunks(ntok):
            blk = (
                xg[c0 : c0 + cw, :].T.reshape(NDB, 128, cw)
                .transpose(1, 0, 2)
                .reshape(128, NDB * cw)
            )
            xc[:, NDB * c0 : NDB * (c0 + cw)] = blk
        xTs.append(xc.astype(xnp))
        if fp8:
            xTbs.append(xc.astype(ml_dtypes.bfloat16))
        bm = np.zeros(ntok, np.float32)
        bm[n:] = -1e30
        biasms.append(np.ascontiguousarray(bm.reshape(n_tb, 128).T))

    # weights: [3072, 1024] -> per (half, proj): [128, NDB, ECOLS]
    w4 = w_qkv.reshape(N_HEADS, D_HEAD, 3, D_MODEL)
    wT = {}
    for half in (0, 1):
        h0 = half * HPC
        for ci, cn in ((0, "q"), (1, "k"), (2, "v")):
            wdt = ml_dtypes.bfloat16 if cn == "v" else xnp
            wsc = 1.0 if cn == "v" else ws
            w2 = w4[h0 : h0 + HPC, :, ci, :].reshape(ECOLS, D_MODEL) * wsc
            # [cols, d] -> [p, db, cols] with d = db*128 + p
            wT[(half, cn)] = np.ascontiguousarray(
                w2.T.reshape(NDB, 128, ECOLS).transpose(1, 0, 2)
            ).astype(wdt)

    in_maps = []
    for c in range(NCORES):
        b, half = divmod(c, 2)
        h0 = half * HPC
        mk = (
            mem_kv[0, h0 : h0 + HPC]
            .reshape(NPAIR, 2, NMEM, D_HEAD)
            .transpose(0, 2, 1, 3)
            .reshape(NPAIR, NMEM, 128)
        )
        mv = (
            mem_kv[1, h0 : h0 + HPC]
            .reshape(NPAIR, 2, NMEM, D_HEAD)
            .transpose(0, 2, 1, 3)
            .reshape(NPAIR, NMEM, 128)
        )
        mvp = np.ones((NPAIR, NMEM, 130), np.float32)
        mvp[:, :, :128] = mv
        im = {
            "xT": xTs[b],
            "wq": wT[(half, "q")],
            "wk": wT[(half, "k")],
            "wv": wT[(half, "v")],
            "mkp": np.exp(mk).astype(ml_dtypes.bfloat16),
            "mvp": mvp.astype(ml_dtypes.bfloat16),
            "biasm": biasms[b],
        }
        if fp8:
            im["xTb"] = xTbs[b]
        in_maps.append(im)
    return in_maps, idxs


def _assemble(results, idxs, nb=B, seqlen=L):
    """Divide num/den, transpose, scatter into the full (b, l, d) output."""
    out = np.zeros((nb, seqlen, D_MODEL), np.float32)
    for c in range(NCORES):
        b, half = divmod(c, 2)
        idx = idxs[b]
        n = len(idx)
        oTp = np.asarray(results[c]["outT"]).astype(np.float32)  # [128, 4*ntok] packed
        ntok = oTp.shape[1] // NPAIR
        oT = np.empty((NPAIR, 128, ntok), np.float32)
        for c0, cw in _chunks(ntok):
            blk = oTp[:, NPAIR * c0 : NPAIR * (c0 + cw)].reshape(128, NPAIR, cw)
            oT[:, :, c0 : c0 + cw] = blk.transpose(1, 0, 2)
        eq = np.asarray(results[c]["expqd"]).astype(np.float32)  # [128, 4, ntok]
        # den[g, h, t] = sum_d expq[h*64+d, g, t]
        den = eq[:, :, :n].reshape(2, 64, NPAIR, n).sum(axis=1)  # [2, 4, n]
        num = oT[:, :, :n].reshape(NPAIR, 2, 64, n)
        y = num / den.transpose(1, 0, 2)[:, :, None, :]
        y = y.transpose(3, 0, 1, 2).reshape(n, ECOLS)
        out[b, idx, half * ECOLS : (half + 1) * ECOLS] = y
    return out


def _get_nc(ntok=NTOK, mode=MODE):
    key = (ntok, mode)
    if key not in _CACHE:
        _CACHE[key] = build_nc(ntok, mode)
    return _CACHE[key]


def kernel(x, w_qkv, mem_kv, mask):
    from concourse.bass_utils import run_bass_kernel_spmd

    mask = np.asarray(mask)
    ntok = NTOK
    max_n = int(mask.sum(axis=1).max())
    if max_n > ntok:  # safety net for unexpected mask densities
        ntok = -(-max_n // 128) * 128
    nc = _get_nc(ntok)
    in_maps, idxs = _host_inputs(x, w_qkv, mem_kv, mask, ntok=ntok)
    res = run_bass_kernel_spmd(nc, in_maps, core_ids=list(range(NCORES)))
    return _assemble(res.results, idxs, nb=x.shape[0], seqlen=x.shape[1])
